# revision 47
# baseline (speedup 1.0000x reference)
"""Multi-head causal attention (B=4, S=2048, D=1024, H=16) on 8 TRN2 NeuronCores.

Sharding: core c handles batch b = c//2 and head-group hg = c%2 (8 heads each).
Each core computes Q/K/V projections for its (batch, head-group), causal
attention, and a partial output projection over its 512 head-dims.  The host
sums the two partials per batch and adds b_o.  No collectives.

Device-side layout choices:
  - x is passed transposed (xT [D, S]) so projection matmuls contract over
    partitions directly.
  - Q and K are produced transposed (QT/KT [dq, S]); scores are computed
    transposed (S^T [kpos, q]).  Per head the contraction is only 64 dims, so
    the two heads of a head-pair run as CONCURRENT row-tiled matmuls in the PE
    array (tile_position rows 0-63 / 64-127) writing adjacent PSUM banks of
    one wide [128, 2, 512] score tile -- 2x the score throughput of a padded
    128-contraction matmul, and one 1024-wide exp instruction per kt tile
    (halves the ACT per-instruction overhead).
  - No max-subtraction in softmax: scaled scores are ~N(0,1), exp is safe.
  - AV matmuls are trimmed to [lo:512] on diagonal tiles (PSUM sub-range
    accumulation) so no P zero-fill is needed below the causal block.
  - V is projected from the x tiles already resident in SBUF (x is DMA'd
    exactly once).
  - Softmax denominator comes from ones-columns in the V stationary; the
    reciprocal is exp(-ln(l)) on ACT (one table set, patched below), run
    from an SBUF staging copy so the single-buffered u psum frees fast.
  - Output partials are stored fp16 (halves output DMA); host sums in fp32.

Scheduling: head DMAs ride three queues (sync HWDGE: wq + x blocks; ACT
HWDGE: x block 0, wv, wo; gpsimd SWDGE: wk) so the first projection chain
starts ~1.5us after the preamble.  Scores for pair i+1 interleave with AV
for pair i at kt granularity.  Out-projections are deferred to the last
pairs as per-qt filler chunks, where the PE would otherwise stall on the
ACT engine's exp backlog (the late pairs are exp-heavy and there are no
projection chains left to interleave).
"""

import sys
import os

sys.path.insert(0, "/opt/trn_rl_repo")

import numpy as np

import concourse.bacc as bacc
import concourse.mybir as mybir
import concourse.tile as tile
from concourse.bass_utils import run_bass_kernel_spmd

# The ACT table-load pass resolves each activation to the first table set
# containing it, which puts Exp (exp_and_others) and Ln
# (natural_log_exp_and_others) in different sets and reloads tables at every
# softmax normalization.  Restrict Exp/Ln to the one set that holds both so
# the whole kernel runs off a single table load.
_orig_get_tables = bacc.get_activation_tables


def _patched_tables(arch):
    t = _orig_get_tables(arch)
    for name, fns in t.items():
        if name != "natural_log_exp_and_others":
            fns.discard(mybir.ActivationFunctionType.Exp)
            fns.discard(mybir.ActivationFunctionType.Ln)
    return t


bacc.get_activation_tables = _patched_tables

B, S, D, H = 4, 2048, 1024, 16
DK = D // H          # 64
HH = H // 2          # 8 heads per core
HD = HH * DK         # 512 head-dims per core
N_CORES = 8

F32 = mybir.dt.float32
F16 = mybir.dt.float16

SCALE = 1.0 / np.sqrt(DK)


def build_nc(s=S):
    """Build the per-core SPMD program.  `s` is the sequence length (tunable
    for small-scale simulation; must be a multiple of 512)."""
    assert s % 512 == 0
    n_qb = s // 512          # 512-wide q blocks
    n_t128 = s // 128        # 128-wide token tiles
    n_dt = D // 128          # din tiles (8)
    n_hp = HH // 2           # head pairs (4)

    nc = bacc.Bacc("TRN2", target_bir_lowering=False, debug=False,
                   num_devices=N_CORES)

    xT = nc.dram_tensor("xT", [D, s], F16, kind="ExternalInput")
    wqT = nc.dram_tensor("wqT", [D, HD], F16, kind="ExternalInput")
    wkT = nc.dram_tensor("wkT", [D, HD], F16, kind="ExternalInput")
    wvT = nc.dram_tensor("wvT", [D, HD], F16, kind="ExternalInput")
    woT = nc.dram_tensor("woT", [HD, D], F16, kind="ExternalInput")
    out = nc.dram_tensor("out", [s, D], F16, kind="ExternalOutput")

    with tile.TileContext(nc) as tc:
        with tc.tile_pool(name="persist", bufs=1) as persist, \
             tc.tile_pool(name="wload", bufs=16) as wload, \
             tc.tile_pool(name="xtb", bufs=16) as xtb_pool, \
             tc.tile_pool(name="pT", bufs=24) as pT_pool, \
             tc.tile_pool(name="aoT", bufs=16) as aoT_pool, \
             tc.tile_pool(name="rb", bufs=4) as rb_pool, \
             tc.tile_pool(name="outsb", bufs=3) as out_pool, \
             tc.tile_pool(name="spsum", bufs=2, space="PSUM") as spsum, \
             tc.tile_pool(name="upsum", bufs=2, space="PSUM") as upsum, \
             tc.tile_pool(name="opsum", bufs=2, space="PSUM") as opsum:

            # Persistent SBUF arrays (live for the whole kernel).
            qt_sb = [persist.tile([128, s], F16, tag=f"qt{d}", name=f"qt{d}")
                     for d in range(n_hp)]
            # K^T per head-pair: head 2*hp on rows 0-63, head 2*hp+1 on rows
            # 64-127.  Scores matmuls slice 64-partition stationaries; the two
            # heads run concurrently as row-tiled matmuls.
            kt_sb = [persist.tile([128, s], F16, tag=f"kt{d}", name=f"kt{d}")
                     for d in range(n_hp)]
            # V tiles hold [t, head, 2*dk]: cols 0-63 are V, cols 64-127 are
            # 1.0.  As the AV stationary this makes the matmul emit U^T on
            # psum rows 0-63 and the softmax denominator on rows 64-127.
            v_sb = [persist.tile([128, HH, 2 * DK], F16, tag=f"v{t}", name=f"v{t}")
                    for t in range(n_t128)]
            wo_sb = [persist.tile([128, D], F16, tag=f"wo{d}", name=f"wo{d}")
                     for d in range(n_hp)]
            wv_sb = [persist.tile([128, HD], F16, tag=f"wv{i}", name=f"wv{i}")
                     for i in range(n_dt)]

            w_tiles = {}
            x_tiles = {}

            def dma_x_block(tb, eng=None):
                # Block 0 rides the ACT engine's HWDGE queue (parallel with
                # the weight DMAs on the sync queue, and no exps are queued
                # yet).  Later blocks use the sync queue so DMA issue slots
                # never sit between exps in the ACT FIFO.
                eng = eng or nc.sync
                ts = []
                for i in range(n_dt):
                    t = xtb_pool.tile([128, 512], F16, tag="xtb", name="xtb")
                    eng.dma_start(
                        out=t[:], in_=xT[i * 128:(i + 1) * 128,
                                         tb * 512:(tb + 1) * 512])
                    ts.append(t)
                x_tiles[tb] = ts

            # Head DMAs across three queues: wq (then wo) on the sync HWDGE
            # queue, x block 0 + wv on the ACT HWDGE queue, wk on the gpsimd
            # SWDGE queue.  The first Q chain starts after one wq tile + one
            # x tile (~256KB) and the K chains' weights land in parallel.
            dma_x_block(0, eng=nc.scalar)
            for i in range(n_dt):
                wt = wload.tile([128, HD], F16, tag="w", name="w")
                nc.sync.dma_start(out=wt[:], in_=wqT[i * 128:(i + 1) * 128, :])
                w_tiles[("q", i)] = wt
            for i in range(n_dt):
                wt = wload.tile([128, HD], F16, tag="w", name="w")
                nc.gpsimd.dma_start(out=wt[:], in_=wkT[i * 128:(i + 1) * 128, :])
                w_tiles[("k", i)] = wt
            # Ones columns for the denominators, written once during the DMA
            # head (they are never overwritten; V copies only touch 0:DK).
            for t in range(n_t128):
                nc.vector.memset(v_sb[t][:, :, DK:2 * DK], 1.0)
            for i in range(n_dt):
                nc.scalar.dma_start(out=wv_sb[i][:], in_=wvT[i * 128:(i + 1) * 128, :])
            for d in range(n_hp):
                nc.scalar.dma_start(out=wo_sb[d][:], in_=woT[d * 128:(d + 1) * 128, :])

            def emit_proj_chains(tb, dqs):
                """Q^T and K^T projection chains for one 512-token block and
                the given dq tiles, from the SBUF-resident x block."""
                if tb not in x_tiles:
                    dma_x_block(tb)
                xs = x_tiles[tb]
                for dq in dqs:
                    for wkey, dst in (("q", qt_sb), ("k", kt_sb)):
                        ps = opsum.tile([128, 512], F32, tag="op", name="pp")
                        for i in range(n_dt):
                            nc.tensor.matmul(
                                ps[:],
                                lhsT=w_tiles[(wkey, i)][:, dq * 128:(dq + 1) * 128],
                                rhs=xs[i][:],
                                start=(i == 0), stop=(i == n_dt - 1),
                            )
                        nc.vector.tensor_copy(
                            out=dst[dq][:, tb * 512:(tb + 1) * 512], in_=ps[:])

            def emit_v_chain(t128):
                """V projection for one 128-token tile, from SBUF x tiles."""
                tb = t128 // 4
                if tb not in x_tiles:
                    dma_x_block(tb)
                xs = x_tiles[tb]
                c = (t128 % 4) * 128
                vp = opsum.tile([128, 512], F32, tag="op", name="vp")
                for i in range(n_dt):
                    nc.tensor.matmul(
                        vp[:], lhsT=xs[i][:, c:c + 128], rhs=wv_sb[i][:],
                        start=(i == 0), stop=(i == n_dt - 1),
                    )
                nc.vector.tensor_copy(
                    out=v_sb[t128][:, :, 0:DK],
                    in_=vp[:].rearrange("p (h k) -> p h k", h=HH))

            def emit_score_kt(qb, hp, kt, pT):
                """Scores + exp for one kt tile, both heads of the pair.

                The two heads' matmuls are row-tiled (64-partition
                stationaries at rows 0-63 / 64-127) and run concurrently in
                the PE array, writing the two banks of a wide psum tile."""
                lo = max(kt - 4 * qb, 0) * 128
                sp = spsum.tile([128, 2, 512], F32, tag="sp", name="sp")
                for hh in (0, 1):
                    nc.tensor.matmul(
                        sp[:, hh, lo:512],
                        lhsT=kt_sb[hp][hh * 64:(hh + 1) * 64,
                                       kt * 128:(kt + 1) * 128],
                        rhs=qt_sb[hp][hh * 64:(hh + 1) * 64,
                                      qb * 512 + lo:(qb + 1) * 512],
                        start=True, stop=True,
                    )
                p = pT_pool.tile([128, 2, 512], F16, tag="p", name="p")
                nc.scalar.activation(
                    out=p[:, :, lo:512], in_=sp[:, :, lo:512],
                    func=mybir.ActivationFunctionType.Exp,
                    scale=float(SCALE))
                if kt >= 4 * qb:
                    # zero strict-upper (kpos > q) region of the
                    # diagonal-crossing tile; only the first 128 columns
                    # after lo can be masked.
                    nc.gpsimd.affine_select(
                        out=p[:, :, lo:lo + 128], in_=p[:, :, lo:lo + 128],
                        compare_op=mybir.AluOpType.is_ge,
                        fill=0.0, base=0, channel_multiplier=-1,
                        pattern=[[0, 2], [1, 128]])
                pT[kt] = (p, lo)

            filler_q = []

            def emit_pair(cur, nxt, pT_cur, pT_next, fill=None):
                """Interleave next pair's scores with current pair's AV
                chains at kt granularity: the PE gets AV matmuls to run
                while the ACT engine works through the scores' exps."""
                nkt_cur = 4 * cur[0] + 4 if cur else 0
                nkt_nxt = 4 * nxt[0] + 4 if nxt else 0
                u = {}
                ao = None
                if cur:
                    ao = aoT_pool.tile([128, 512], F16, tag="aoT", name="aoT")
                    for hh in (0, 1):
                        u[hh] = upsum.tile([128, 512], F32, tag="u", name="u")
                for kt in range(max(nkt_cur, nkt_nxt)):
                    if fill and filler_q and kt in fill:
                        filler_q.pop(0)()
                    if kt < nkt_nxt:
                        emit_score_kt(nxt[0], nxt[1], kt, pT_next)
                    if kt < nkt_cur:
                        p, lo = pT_cur[kt]
                        for hh in (0, 1):
                            nc.tensor.matmul(
                                u[hh][:, lo:512],
                                lhsT=v_sb[kt][:, 2 * cur[1] + hh, :],
                                rhs=p[:, hh, lo:512],
                                start=(kt == 0), stop=(kt == nkt_cur - 1),
                                skip_group_check=True,
                            )
                    if cur and kt == nkt_cur - 1:
                        # Normalization immediately after the last AV matmul.
                        # rows 0-63: U^T; rows 64-127: denominator bcast.
                        # 1/l = exp(-ln(l)): ln and exp share one ACT table
                        # set, so no table reloads; the Exp covers both heads'
                        # Ln outputs in one 1024-wide instruction.
                        rb = rb_pool.tile([128, 2, 512], F32, tag="rb", name="rb")
                        if nxt is not None:
                            # Stage u to SBUF with fast DVE copies first: the
                            # u psum tiles (upsum has no double-buffering) are
                            # then released without waiting on the ACT
                            # engine's Ln, which sits behind the exp backlog.
                            uc = rb_pool.tile([128, 2, 512], F32, tag="rb", name="rb")
                            for hh in (0, 1):
                                nc.vector.tensor_copy(out=uc[:, hh, :], in_=u[hh][:])
                            for hh in (0, 1):
                                # Ln lands on partitions 0-63 so the final
                                # mul's SBUF inputs share a base partition.
                                nc.scalar.activation(
                                    out=rb[0:64, hh, :], in_=uc[64:128, hh, :],
                                    func=mybir.ActivationFunctionType.Ln)
                            nc.scalar.activation(
                                out=rb[0:64, :, :], in_=rb[0:64, :, :],
                                func=mybir.ActivationFunctionType.Exp,
                                scale=-1.0)
                            for hh in (0, 1):
                                nc.vector.tensor_mul(
                                    out=ao[hh * 64:(hh + 1) * 64, :],
                                    in0=uc[0:64, hh, :], in1=rb[0:64, hh, :])
                        else:
                            # Final pair: nothing needs the psum released, and
                            # the DVE queue is full of filler-oproj casts --
                            # read u directly, and pipeline per head so ao's
                            # first half is ready one Ln+Exp earlier.
                            for hh in (0, 1):
                                nc.scalar.activation(
                                    out=rb[0:64, hh, :], in_=u[hh][64:128, :],
                                    func=mybir.ActivationFunctionType.Ln)
                                nc.scalar.activation(
                                    out=rb[0:64, hh, :], in_=rb[0:64, hh, :],
                                    func=mybir.ActivationFunctionType.Exp,
                                    scale=-1.0)
                                nc.vector.tensor_mul(
                                    out=ao[hh * 64:(hh + 1) * 64, :],
                                    in0=u[hh][0:64, :], in1=rb[0:64, hh, :])
                return ao

            def emit_oproj(qb, ao_pairs, qt_ls=(0, 1, 2, 3)):
                for qt_l in qt_ls:
                    qt = 4 * qb + qt_l
                    osb = out_pool.tile([128, D], F16, tag="osb", name="osb")
                    for half in range(2):
                        op = opsum.tile([128, 512], F32, tag="op", name="op")
                        for hp in range(n_hp):
                            nc.tensor.matmul(
                                op[:],
                                lhsT=ao_pairs[hp][:, qt_l * 128:(qt_l + 1) * 128],
                                rhs=wo_sb[hp][:, half * 512:(half + 1) * 512],
                                start=(hp == 0), stop=(hp == n_hp - 1),
                            )
                        nc.vector.tensor_copy(
                            out=osb[:, half * 512:(half + 1) * 512], in_=op[:])
                    nc.sync.dma_start(
                        out=out[qt * 128:(qt + 1) * 128, :], in_=osb[:])

            # Demand-driven schedule: projections for q-block tb are emitted
            # inside q-block tb-1's pairs; V chains just before the block
            # needing them; out-projections three pairs after their block.
            # dq-interleaved first block so pair (0,0) unblocks after two
            # chains.
            for dq in range(n_hp):
                emit_proj_chains(0, [dq])
            for t in range(4):
                emit_v_chain(t)
            pairs = [(qb, hp) for qb in range(n_qb) for hp in range(n_hp)]
            n_pairs = len(pairs)
            pT_next = {}
            emit_pair(None, pairs[0], None, pT_next)
            ao_by_qb = {qb: [] for qb in range(n_qb)}

            def make_oproj_chunk(qb, qt_l, ao_pairs):
                return lambda: emit_oproj(qb, ao_pairs, (qt_l,))

            # Out-projections are deferred to the tail of the pair stream,
            # where the proj-chain filler runs out and the PE would otherwise
            # stall on the ACT engine's exp backlog (the late pairs are
            # exp-heavy).  They are emitted as per-qt chunks inside the last
            # pairs' kt loops.
            for i, (qb, hp) in enumerate(pairs):
                pT_cur, pT_next = pT_next, {}
                nxt = pairs[i + 1] if i + 1 < len(pairs) else None
                if nxt and nxt[1] == 0 and nxt[0] > 0:
                    for t in range(4 * nxt[0], 4 * nxt[0] + 4):
                        emit_v_chain(t)
                # Filler kt-slots, weighted toward the latest pairs where the
                # ACT exp backlog (and thus the PE stall) is deepest.
                fill_kts = {
                    n_pairs - 4: (4, 10),
                    n_pairs - 3: (4, 9, 14),
                    n_pairs - 2: (4, 9, 14),
                    n_pairs - 1: (3, 7, 11, 15),
                }.get(i)
                ao_by_qb[qb].append(emit_pair((qb, hp), nxt, pT_cur, pT_next,
                                              fill=fill_kts))
                if qb + 1 < n_qb:
                    # projections for the next q-block, one dq chain per pair
                    emit_proj_chains(qb + 1, [hp])
                if hp == n_hp - 1:
                    aop = list(ao_by_qb[qb])
                    for qt_l in range(4):
                        filler_q.append(make_oproj_chunk(qb, qt_l, aop))
            for f in filler_q:
                f()

    nc.compile()
    return nc


_NC_CACHE = {}


def _get_nc(s=S):
    if s not in _NC_CACHE:
        _NC_CACHE[s] = build_nc(s)
    return _NC_CACHE[s]


def make_in_maps(x, w_q, w_k, w_v, w_o, s=S):
    """Host-side sharding: returns the 8 per-core input maps."""
    x = np.ascontiguousarray(np.asarray(x, dtype=np.float32))
    w_q = np.asarray(w_q, dtype=np.float32)
    w_k = np.asarray(w_k, dtype=np.float32)
    w_v = np.asarray(w_v, dtype=np.float32)
    w_o = np.asarray(w_o, dtype=np.float32)

    xTs = [np.ascontiguousarray(x[b].T.astype(np.float16)) for b in range(B)]
    wqTs = [np.ascontiguousarray(w_q[hg * HD:(hg + 1) * HD, :].T.astype(np.float16)) for hg in range(2)]
    wkTs = [np.ascontiguousarray(w_k[hg * HD:(hg + 1) * HD, :].T.astype(np.float16)) for hg in range(2)]
    wvTs = [np.ascontiguousarray(w_v[hg * HD:(hg + 1) * HD, :].T.astype(np.float16)) for hg in range(2)]
    woTs = [np.ascontiguousarray(w_o[:, hg * HD:(hg + 1) * HD].T.astype(np.float16)) for hg in range(2)]

    in_maps = []
    for c in range(N_CORES):
        b, hg = c // 2, c % 2
        in_maps.append({
            "xT": xTs[b], "wqT": wqTs[hg], "wkT": wkTs[hg],
            "wvT": wvTs[hg], "woT": woTs[hg],
        })
    return in_maps


def kernel(x, w_q, w_k, w_v, w_o, b_o):
    nc = _get_nc(S)
    in_maps = make_in_maps(x, w_q, w_k, w_v, w_o, s=S)
    res = run_bass_kernel_spmd(nc, in_maps, core_ids=list(range(N_CORES)))
    b_o = np.asarray(b_o, dtype=np.float32)
    outp = np.empty((B, S, D), dtype=np.float32)
    for b in range(B):
        outp[b] = (res.results[2 * b]["out"].astype(np.float32)
                   + res.results[2 * b + 1]["out"].astype(np.float32) + b_o)
    return outp


# revision 49
# speedup vs baseline: 1.0095x; 1.0095x over previous
"""Multi-head causal attention (B=4, S=2048, D=1024, H=16) on 8 TRN2 NeuronCores.

Sharding: core c handles batch b = c//2 and head-group hg = c%2 (8 heads each).
Each core computes Q/K/V projections for its (batch, head-group), causal
attention, and a partial output projection over its 512 head-dims.  The host
sums the two partials per batch and adds b_o.  No collectives.

Device-side layout choices:
  - x is passed transposed (xT [D, S]) so projection matmuls contract over
    partitions directly.
  - Q and K are produced transposed (QT/KT [dq, S]); scores are computed
    transposed (S^T [kpos, q]).  Per head the contraction is only 64 dims, so
    the two heads of a head-pair run as CONCURRENT row-tiled matmuls in the PE
    array (tile_position rows 0-63 / 64-127) writing adjacent PSUM banks of
    one wide [128, 2, 512] score tile -- 2x the score throughput of a padded
    128-contraction matmul, and one 1024-wide exp instruction per kt tile
    (halves the ACT per-instruction overhead).
  - No max-subtraction in softmax: scaled scores are ~N(0,1), exp is safe.
  - AV matmuls are trimmed to [lo:512] on diagonal tiles (PSUM sub-range
    accumulation) so no P zero-fill is needed below the causal block.
  - V is projected from the x tiles already resident in SBUF (x is DMA'd
    exactly once).
  - Softmax denominator comes from ones-columns in the V stationary; the
    reciprocal is exp(-ln(l)) on ACT (one table set, patched below), run
    from an SBUF staging copy so the single-buffered u psum frees fast.
  - Output partials are stored fp16 (halves output DMA); host sums in fp32.

Scheduling: head DMAs ride three queues (sync HWDGE: wq + x blocks; ACT
HWDGE: x block 0, wv, wo; gpsimd SWDGE: wk) so the first projection chain
starts ~1.5us after the preamble.  Scores for pair i+1 interleave with AV
for pair i at kt granularity.  Out-projections are deferred to the last
pairs as per-qt filler chunks, where the PE would otherwise stall on the
ACT engine's exp backlog (the late pairs are exp-heavy and there are no
projection chains left to interleave).
"""

import sys
import os

sys.path.insert(0, "/opt/trn_rl_repo")

import numpy as np

import concourse.bacc as bacc
import concourse.mybir as mybir
import concourse.tile as tile
from concourse.bass_utils import run_bass_kernel_spmd

# The ACT table-load pass resolves each activation to the first table set
# containing it, which puts Exp (exp_and_others) and Ln
# (natural_log_exp_and_others) in different sets and reloads tables at every
# softmax normalization.  Restrict Exp/Ln to the one set that holds both so
# the whole kernel runs off a single table load.
_orig_get_tables = bacc.get_activation_tables


def _patched_tables(arch):
    t = _orig_get_tables(arch)
    for name, fns in t.items():
        if name != "natural_log_exp_and_others":
            fns.discard(mybir.ActivationFunctionType.Exp)
            fns.discard(mybir.ActivationFunctionType.Ln)
    return t


bacc.get_activation_tables = _patched_tables

B, S, D, H = 4, 2048, 1024, 16
DK = D // H          # 64
HH = H // 2          # 8 heads per core
HD = HH * DK         # 512 head-dims per core
N_CORES = 8

F32 = mybir.dt.float32
F16 = mybir.dt.float16

SCALE = 1.0 / np.sqrt(DK)


def build_nc(s=S):
    """Build the per-core SPMD program.  `s` is the sequence length (tunable
    for small-scale simulation; must be a multiple of 512)."""
    assert s % 512 == 0
    n_qb = s // 512          # 512-wide q blocks
    n_t128 = s // 128        # 128-wide token tiles
    n_dt = D // 128          # din tiles (8)
    n_hp = HH // 2           # head pairs (4)

    nc = bacc.Bacc("TRN2", target_bir_lowering=False, debug=False,
                   num_devices=N_CORES)

    xT = nc.dram_tensor("xT", [D, s], F16, kind="ExternalInput")
    wqT = nc.dram_tensor("wqT", [D, HD], F16, kind="ExternalInput")
    wkT = nc.dram_tensor("wkT", [D, HD], F16, kind="ExternalInput")
    wvT = nc.dram_tensor("wvT", [D, HD], F16, kind="ExternalInput")
    woT = nc.dram_tensor("woT", [HD, D], F16, kind="ExternalInput")
    out = nc.dram_tensor("out", [s, D], F16, kind="ExternalOutput")

    with tile.TileContext(nc) as tc:
        with tc.tile_pool(name="persist", bufs=1) as persist, \
             tc.tile_pool(name="wload", bufs=16) as wload, \
             tc.tile_pool(name="xtb", bufs=16) as xtb_pool, \
             tc.tile_pool(name="pT", bufs=24) as pT_pool, \
             tc.tile_pool(name="aoT", bufs=16) as aoT_pool, \
             tc.tile_pool(name="rb", bufs=4) as rb_pool, \
             tc.tile_pool(name="outsb", bufs=3) as out_pool, \
             tc.tile_pool(name="spsum", bufs=2, space="PSUM") as spsum, \
             tc.tile_pool(name="upsum", bufs=2, space="PSUM") as upsum, \
             tc.tile_pool(name="opsum", bufs=2, space="PSUM") as opsum:

            # Persistent SBUF arrays (live for the whole kernel).
            qt_sb = [persist.tile([128, s], F16, tag=f"qt{d}", name=f"qt{d}")
                     for d in range(n_hp)]
            # K^T per head-pair: head 2*hp on rows 0-63, head 2*hp+1 on rows
            # 64-127.  Scores matmuls slice 64-partition stationaries; the two
            # heads run concurrently as row-tiled matmuls.
            kt_sb = [persist.tile([128, s], F16, tag=f"kt{d}", name=f"kt{d}")
                     for d in range(n_hp)]
            # V tiles hold [t, head, 2*dk]: cols 0-63 are V, cols 64-127 are
            # 1.0.  As the AV stationary this makes the matmul emit U^T on
            # psum rows 0-63 and the softmax denominator on rows 64-127.
            v_sb = [persist.tile([128, HH, 2 * DK], F16, tag=f"v{t}", name=f"v{t}")
                    for t in range(n_t128)]
            wo_sb = [persist.tile([128, D], F16, tag=f"wo{d}", name=f"wo{d}")
                     for d in range(n_hp)]
            wv_sb = [persist.tile([128, HD], F16, tag=f"wv{i}", name=f"wv{i}")
                     for i in range(n_dt)]

            w_tiles = {}
            x_tiles = {}

            def dma_x_block(tb, eng=None):
                # Block 0 rides the ACT engine's HWDGE queue (parallel with
                # the weight DMAs on the sync queue, and no exps are queued
                # yet).  Later blocks use the sync queue so DMA issue slots
                # never sit between exps in the ACT FIFO.
                eng = eng or nc.sync
                ts = []
                for i in range(n_dt):
                    t = xtb_pool.tile([128, 512], F16, tag="xtb", name="xtb")
                    eng.dma_start(
                        out=t[:], in_=xT[i * 128:(i + 1) * 128,
                                         tb * 512:(tb + 1) * 512])
                    ts.append(t)
                x_tiles[tb] = ts

            # Head DMAs across three queues: wq (then wo) on the sync HWDGE
            # queue, x block 0 + wv on the ACT HWDGE queue, wk on the gpsimd
            # SWDGE queue.  The first Q chain starts after one wq tile + one
            # x tile (~256KB) and the K chains' weights land in parallel.
            dma_x_block(0, eng=nc.scalar)
            for i in range(n_dt):
                wt = wload.tile([128, HD], F16, tag="w", name="w")
                nc.sync.dma_start(out=wt[:], in_=wqT[i * 128:(i + 1) * 128, :])
                w_tiles[("q", i)] = wt
            for i in range(n_dt):
                wt = wload.tile([128, HD], F16, tag="w", name="w")
                nc.gpsimd.dma_start(out=wt[:], in_=wkT[i * 128:(i + 1) * 128, :])
                w_tiles[("k", i)] = wt
            # Ones columns for the denominators, written once during the DMA
            # head (they are never overwritten; V copies only touch 0:DK).
            for t in range(n_t128):
                nc.vector.memset(v_sb[t][:, :, DK:2 * DK], 1.0)
            for i in range(n_dt):
                nc.scalar.dma_start(out=wv_sb[i][:], in_=wvT[i * 128:(i + 1) * 128, :])
            for d in range(n_hp):
                nc.scalar.dma_start(out=wo_sb[d][:], in_=woT[d * 128:(d + 1) * 128, :])

            def emit_proj_chains(tb, dqs):
                """Q^T and K^T projection chains for one 512-token block and
                the given dq tiles, from the SBUF-resident x block."""
                if tb not in x_tiles:
                    dma_x_block(tb)
                xs = x_tiles[tb]
                for dq in dqs:
                    for wkey, dst in (("q", qt_sb), ("k", kt_sb)):
                        ps = opsum.tile([128, 512], F32, tag="op", name="pp")
                        for i in range(n_dt):
                            nc.tensor.matmul(
                                ps[:],
                                lhsT=w_tiles[(wkey, i)][:, dq * 128:(dq + 1) * 128],
                                rhs=xs[i][:],
                                start=(i == 0), stop=(i == n_dt - 1),
                            )
                        nc.vector.tensor_copy(
                            out=dst[dq][:, tb * 512:(tb + 1) * 512], in_=ps[:])

            def emit_v_chain(t128):
                """V projection for one 128-token tile, from SBUF x tiles."""
                tb = t128 // 4
                if tb not in x_tiles:
                    dma_x_block(tb)
                xs = x_tiles[tb]
                c = (t128 % 4) * 128
                vp = opsum.tile([128, 512], F32, tag="op", name="vp")
                for i in range(n_dt):
                    nc.tensor.matmul(
                        vp[:], lhsT=xs[i][:, c:c + 128], rhs=wv_sb[i][:],
                        start=(i == 0), stop=(i == n_dt - 1),
                    )
                nc.vector.tensor_copy(
                    out=v_sb[t128][:, :, 0:DK],
                    in_=vp[:].rearrange("p (h k) -> p h k", h=HH))

            def emit_score_kt(qb, hp, kt, pT):
                """Scores + exp for one kt tile, both heads of the pair.

                The two heads' matmuls are row-tiled (64-partition
                stationaries at rows 0-63 / 64-127) and run concurrently in
                the PE array, writing the two banks of a wide psum tile."""
                lo = max(kt - 4 * qb, 0) * 128
                sp = spsum.tile([128, 2, 512], F32, tag="sp", name="sp")
                for hh in (0, 1):
                    nc.tensor.matmul(
                        sp[:, hh, lo:512],
                        lhsT=kt_sb[hp][hh * 64:(hh + 1) * 64,
                                       kt * 128:(kt + 1) * 128],
                        rhs=qt_sb[hp][hh * 64:(hh + 1) * 64,
                                      qb * 512 + lo:(qb + 1) * 512],
                        start=True, stop=True,
                    )
                p = pT_pool.tile([128, 2, 512], F16, tag="p", name="p")
                nc.scalar.activation(
                    out=p[:, :, lo:512], in_=sp[:, :, lo:512],
                    func=mybir.ActivationFunctionType.Exp,
                    scale=float(SCALE))
                if kt >= 4 * qb:
                    # zero strict-upper (kpos > q) region of the
                    # diagonal-crossing tile; only the first 128 columns
                    # after lo can be masked.
                    nc.gpsimd.affine_select(
                        out=p[:, :, lo:lo + 128], in_=p[:, :, lo:lo + 128],
                        compare_op=mybir.AluOpType.is_ge,
                        fill=0.0, base=0, channel_multiplier=-1,
                        pattern=[[0, 2], [1, 128]])
                pT[kt] = (p, lo)

            filler_q = []

            def emit_pair(cur, nxt, pT_cur, pT_next, fill=None):
                """Interleave next pair's scores with current pair's AV
                chains at kt granularity: the PE gets AV matmuls to run
                while the ACT engine works through the scores' exps."""
                nkt_cur = 4 * cur[0] + 4 if cur else 0
                nkt_nxt = 4 * nxt[0] + 4 if nxt else 0
                u = {}
                ao = None
                if cur:
                    ao = aoT_pool.tile([128, 512], F16, tag="aoT", name="aoT")
                    for hh in (0, 1):
                        u[hh] = upsum.tile([128, 512], F32, tag="u", name="u")
                for kt in range(max(nkt_cur, nkt_nxt)):
                    if fill and filler_q and kt in fill:
                        filler_q.pop(0)()
                    if kt < nkt_nxt:
                        emit_score_kt(nxt[0], nxt[1], kt, pT_next)
                    if kt < nkt_cur:
                        p, lo = pT_cur[kt]
                        for hh in (0, 1):
                            nc.tensor.matmul(
                                u[hh][:, lo:512],
                                lhsT=v_sb[kt][:, 2 * cur[1] + hh, :],
                                rhs=p[:, hh, lo:512],
                                start=(kt == 0), stop=(kt == nkt_cur - 1),
                                skip_group_check=True,
                            )
                    if cur and kt == nkt_cur - 1:
                        # Normalization immediately after the last AV matmul.
                        # rows 0-63: U^T; rows 64-127: denominator bcast.
                        # 1/l = exp(-ln(l)): ln and exp share one ACT table
                        # set, so no table reloads; the Exp covers both heads'
                        # Ln outputs in one 1024-wide instruction.
                        rb = rb_pool.tile([128, 2, 512], F32, tag="rb", name="rb")
                        if nxt is not None:
                            # Stage u to SBUF with fast DVE copies first: the
                            # u psum tiles (upsum has no double-buffering) are
                            # then released without waiting on the ACT
                            # engine's Ln, which sits behind the exp backlog.
                            uc = rb_pool.tile([128, 2, 512], F32, tag="rb", name="rb")
                            for hh in (0, 1):
                                nc.vector.tensor_copy(out=uc[:, hh, :], in_=u[hh][:])
                            for hh in (0, 1):
                                # Ln lands on partitions 0-63 so the final
                                # mul's SBUF inputs share a base partition.
                                nc.scalar.activation(
                                    out=rb[0:64, hh, :], in_=uc[64:128, hh, :],
                                    func=mybir.ActivationFunctionType.Ln)
                            nc.scalar.activation(
                                out=rb[0:64, :, :], in_=rb[0:64, :, :],
                                func=mybir.ActivationFunctionType.Exp,
                                scale=-1.0)
                            for hh in (0, 1):
                                nc.vector.tensor_mul(
                                    out=ao[hh * 64:(hh + 1) * 64, :],
                                    in0=uc[0:64, hh, :], in1=rb[0:64, hh, :])
                        else:
                            # Final pair: nothing needs the psum released, and
                            # the DVE queue is full of filler-oproj casts --
                            # read u directly, and pipeline per head so ao's
                            # first half is ready one Ln+Exp earlier.
                            for hh in (0, 1):
                                nc.scalar.activation(
                                    out=rb[0:64, hh, :], in_=u[hh][64:128, :],
                                    func=mybir.ActivationFunctionType.Ln)
                                nc.scalar.activation(
                                    out=rb[0:64, hh, :], in_=rb[0:64, hh, :],
                                    func=mybir.ActivationFunctionType.Exp,
                                    scale=-1.0)
                                nc.vector.tensor_mul(
                                    out=ao[hh * 64:(hh + 1) * 64, :],
                                    in0=u[hh][0:64, :], in1=rb[0:64, hh, :])
                return ao

            def emit_oproj(qb, ao_pairs, qt_ls=(0, 1, 2, 3), use_spsum=False):
                # use_spsum: after the final pair no more scores are emitted,
                # so the 4 score-psum banks are free; borrowing them lets
                # three qt chunks' chains run concurrently with the final
                # pair's normalization instead of two (opsum has 2 bufs).
                for qt_l in qt_ls:
                    qt = 4 * qb + qt_l
                    osb = out_pool.tile([128, D], F16, tag="osb", name="osb")
                    wide = (spsum.tile([128, 2, 512], F32, tag="sp", name="sp")
                            if use_spsum else None)
                    for half in range(2):
                        op = (wide[:, half, :] if use_spsum else
                              opsum.tile([128, 512], F32, tag="op", name="op")[:])
                        for hp in range(n_hp):
                            nc.tensor.matmul(
                                op,
                                lhsT=ao_pairs[hp][:, qt_l * 128:(qt_l + 1) * 128],
                                rhs=wo_sb[hp][:, half * 512:(half + 1) * 512],
                                start=(hp == 0), stop=(hp == n_hp - 1),
                            )
                        nc.vector.tensor_copy(
                            out=osb[:, half * 512:(half + 1) * 512], in_=op)
                    nc.sync.dma_start(
                        out=out[qt * 128:(qt + 1) * 128, :], in_=osb[:])

            # Demand-driven schedule: projections for q-block tb are emitted
            # inside q-block tb-1's pairs; V chains just before the block
            # needing them; out-projections three pairs after their block.
            # dq-interleaved first block so pair (0,0) unblocks after two
            # chains.
            for dq in range(n_hp):
                emit_proj_chains(0, [dq])
            for t in range(4):
                emit_v_chain(t)
            pairs = [(qb, hp) for qb in range(n_qb) for hp in range(n_hp)]
            n_pairs = len(pairs)
            pT_next = {}
            emit_pair(None, pairs[0], None, pT_next)
            ao_by_qb = {qb: [] for qb in range(n_qb)}

            def make_oproj_chunk(qb, qt_l, ao_pairs):
                return lambda: emit_oproj(qb, ao_pairs, (qt_l,))

            # Out-projections are deferred to the tail of the pair stream,
            # where the proj-chain filler runs out and the PE would otherwise
            # stall on the ACT engine's exp backlog (the late pairs are
            # exp-heavy).  They are emitted as per-qt chunks inside the last
            # pairs' kt loops.
            for i, (qb, hp) in enumerate(pairs):
                pT_cur, pT_next = pT_next, {}
                nxt = pairs[i + 1] if i + 1 < len(pairs) else None
                if nxt and nxt[1] == 0 and nxt[0] > 0:
                    for t in range(4 * nxt[0], 4 * nxt[0] + 4):
                        emit_v_chain(t)
                # Filler kt-slots, weighted toward the latest pairs where the
                # ACT exp backlog (and thus the PE stall) is deepest.
                fill_kts = {
                    n_pairs - 4: (4, 10),
                    n_pairs - 3: (4, 9, 14),
                    n_pairs - 2: (4, 9, 14),
                    n_pairs - 1: (3, 7, 11, 15),
                }.get(i)
                ao_by_qb[qb].append(emit_pair((qb, hp), nxt, pT_cur, pT_next,
                                              fill=fill_kts))
                if qb + 1 < n_qb:
                    # projections for the next q-block, one dq chain per pair
                    emit_proj_chains(qb + 1, [hp])
                if hp == n_hp - 1:
                    aop = list(ao_by_qb[qb])
                    for qt_l in range(4):
                        filler_q.append(make_oproj_chunk(qb, qt_l, aop))
            # Post-loop chunks (the final q-block): alternate the borrowed
            # score-psum banks with opsum so six half-chains' hp0..2 matmuls
            # can run while the final normalization completes.
            aop = list(ao_by_qb[n_qb - 1])
            leftover = [f for f in filler_q]
            if len(leftover) == 4:
                emit_oproj(n_qb - 1, aop, (0,), use_spsum=True)
                emit_oproj(n_qb - 1, aop, (2,))
                emit_oproj(n_qb - 1, aop, (1,), use_spsum=True)
                emit_oproj(n_qb - 1, aop, (3,))
            else:
                for f in leftover:
                    f()

    nc.compile()
    return nc


_NC_CACHE = {}


def _get_nc(s=S):
    if s not in _NC_CACHE:
        _NC_CACHE[s] = build_nc(s)
    return _NC_CACHE[s]


def make_in_maps(x, w_q, w_k, w_v, w_o, s=S):
    """Host-side sharding: returns the 8 per-core input maps."""
    x = np.ascontiguousarray(np.asarray(x, dtype=np.float32))
    w_q = np.asarray(w_q, dtype=np.float32)
    w_k = np.asarray(w_k, dtype=np.float32)
    w_v = np.asarray(w_v, dtype=np.float32)
    w_o = np.asarray(w_o, dtype=np.float32)

    xTs = [np.ascontiguousarray(x[b].T.astype(np.float16)) for b in range(B)]
    wqTs = [np.ascontiguousarray(w_q[hg * HD:(hg + 1) * HD, :].T.astype(np.float16)) for hg in range(2)]
    wkTs = [np.ascontiguousarray(w_k[hg * HD:(hg + 1) * HD, :].T.astype(np.float16)) for hg in range(2)]
    wvTs = [np.ascontiguousarray(w_v[hg * HD:(hg + 1) * HD, :].T.astype(np.float16)) for hg in range(2)]
    woTs = [np.ascontiguousarray(w_o[:, hg * HD:(hg + 1) * HD].T.astype(np.float16)) for hg in range(2)]

    in_maps = []
    for c in range(N_CORES):
        b, hg = c // 2, c % 2
        in_maps.append({
            "xT": xTs[b], "wqT": wqTs[hg], "wkT": wkTs[hg],
            "wvT": wvTs[hg], "woT": woTs[hg],
        })
    return in_maps


def kernel(x, w_q, w_k, w_v, w_o, b_o):
    nc = _get_nc(S)
    in_maps = make_in_maps(x, w_q, w_k, w_v, w_o, s=S)
    res = run_bass_kernel_spmd(nc, in_maps, core_ids=list(range(N_CORES)))
    b_o = np.asarray(b_o, dtype=np.float32)
    outp = np.empty((B, S, D), dtype=np.float32)
    for b in range(B):
        outp[b] = (res.results[2 * b]["out"].astype(np.float32)
                   + res.results[2 * b + 1]["out"].astype(np.float32) + b_o)
    return outp
